# revision 28
# baseline (speedup 1.0000x reference)
"""Trainium2 Bass kernel for nn_BoundsReLUWrapper (CROWN ReLU relaxation).

Math (validated against the jax reference):
  ssq_l[l,do] = sum_di lw[l,di,do]^2 ; ssq_u likewise
  l = lb - eps*sqrt(ssq_l) ; u = ub + eps*sqrt(ssq_u)
  mask_pos = (l>0); mask_neg = (u<0); mask_both = 1 - mask_pos - mask_neg
  k_u = u / (u - l + 1e-12); kp_u = (k_u>0); k_l = (|u|>|l|)
  a = mask_pos + mask_both*k_l          -> lw_o = a*lw
  b = mask_pos + (mask_both*kp_u)*k_u   -> uw_o = b*uw
  lb_o = mask_pos*lb + (mask_both*k_l)*lb
  ub_o = mask_pos*ub + mask_both*(kp_u*((ub-l)*k_u))
  (the kn_u branches are exactly zero for all real inputs)

Sharding: 8 cores, each takes 32 of the 256 "length" rows (embarrassingly
parallel; the di-reduction stays local).

Per-core layout: partition p = l_local*4 + dh  (di = dh*64 + dl),
free f = dl*256 + do.  The per-core shard [32,256,256] is then an exact
reshape to [128, 16384], the DVE reduces over dl via axis=X, and one
fp32 matmul against a 0/1 selection matrix finishes the 4-way dh
reduction (another broadcasts the [32,do] coefficients back to [128,do]).

The do axis is split into NS=2 independent halves so the first half's
scale+store overlaps the second half's loads (hides the coefficient
barrier).  Loads ride the SP HWDGE ring (emitted first); stores ride the
ACT ring so their compute-waits never stall load issue — HW-measured
fastest arrangement of the ones tried.
"""

import sys

for _p in ("/opt/trn_rl_repo", "/root/.axon_site/_ro/trn_rl_repo"):
    if _p not in sys.path:
        sys.path.append(_p)

import numpy as np

EPS = 1e-12  # epsilon from the source module

N_CORES = 8
L = 256
L_LOC = L // N_CORES  # 32
DI = 256
DO = 256
DH = 4                # di = dh*64 + dl
DL = DI // DH         # 64
P = L_LOC * DH        # 128 partitions
FREE = DL * DO        # 16384 elements per partition

NS = 2                # independent do-splits
CCH = 4               # chunks (dl blocks) per split per input (1 MiB DMAs)
STORE_RING = "scalar"  # HWDGE ring for stores: "scalar" (ACT) | "sync" (SP)


def _build_nc(eps_s: float, reps: int = 1, ns: int = None, cch: int = None,
              store_ring: str = None):
    """reps>1 wraps the whole body in a hardware loop (benchmarking only)."""
    import concourse.bacc as bacc
    import concourse.tile as tile
    import concourse.mybir as mybir
    from contextlib import ExitStack, nullcontext

    NS = ns if ns is not None else globals()["NS"]
    CCH = cch if cch is not None else globals()["CCH"]
    DOS = DO // NS
    DLC = DL // CCH
    FC = DLC * DOS

    f32 = mybir.dt.float32
    Alu = mybir.AluOpType

    nc = bacc.Bacc("TRN2", target_bir_lowering=False, debug=False)

    lw_in = nc.dram_tensor("lw_in", [P, FREE], f32, kind="ExternalInput")
    uw_in = nc.dram_tensor("uw_in", [P, FREE], f32, kind="ExternalInput")
    lb_in = nc.dram_tensor("lb_in", [L_LOC, DO], f32, kind="ExternalInput")
    ub_in = nc.dram_tensor("ub_in", [L_LOC, DO], f32, kind="ExternalInput")
    # sel1[k, m] = 1.0 if k//DH == m else 0  (dh-reduction), [P, L_LOC]
    sel1_in = nc.dram_tensor("sel1_in", [P, L_LOC], f32, kind="ExternalInput")
    # sel2[k, m] = 1.0 if k == m//DH else 0  (broadcast), [L_LOC, P]
    sel2_in = nc.dram_tensor("sel2_in", [L_LOC, P], f32, kind="ExternalInput")

    lw_out = nc.dram_tensor("lw_out", [P, FREE], f32, kind="ExternalOutput")
    uw_out = nc.dram_tensor("uw_out", [P, FREE], f32, kind="ExternalOutput")
    lb_out = nc.dram_tensor("lb_out", [L_LOC, DO], f32, kind="ExternalOutput")
    ub_out = nc.dram_tensor("ub_out", [L_LOC, DO], f32, kind="ExternalOutput")

    # [p, dl, do] views of the big DRAM tensors
    lw_iv = lw_in[:, :].rearrange("p (dl do) -> p dl do", do=DO)
    uw_iv = uw_in[:, :].rearrange("p (dl do) -> p dl do", do=DO)
    lw_ov = lw_out[:, :].rearrange("p (dl do) -> p dl do", do=DO)
    uw_ov = uw_out[:, :].rearrange("p (dl do) -> p dl do", do=DO)

    with tile.TileContext(nc) as tc, ExitStack() as ctx:
        bigp = ctx.enter_context(tc.tile_pool(name="bigp", bufs=1))
        sqp = ctx.enter_context(tc.tile_pool(name="sqp", bufs=2))
        partp = ctx.enter_context(tc.tile_pool(name="partp", bufs=4))
        coefp = ctx.enter_context(tc.tile_pool(name="coefp", bufs=NS))
        psp = ctx.enter_context(tc.psum_pool(name="psp", bufs=1))

        store_eng = nc.scalar if (store_ring or STORE_RING) == "scalar" else nc.sync

        def emit_smalls():
            sel1_t = coefp.tile([P, L_LOC], f32, name="sel1_t", bufs=1)
            nc.sync.dma_start(out=sel1_t, in_=sel1_in[:, :])
            sel2_t = coefp.tile([L_LOC, P], f32, name="sel2_t", bufs=1)
            nc.sync.dma_start(out=sel2_t, in_=sel2_in[:, :])
            lb_t = coefp.tile([L_LOC, DO], f32, name="lb_t", bufs=1)
            nc.sync.dma_start(out=lb_t, in_=lb_in[:, :])
            ub_t = coefp.tile([L_LOC, DO], f32, name="ub_t", bufs=1)
            nc.sync.dma_start(out=ub_t, in_=ub_in[:, :])
            return sel1_t, sel2_t, lb_t, ub_t

        # split tiles, [128, (dl, do_half)] packed; scaled in place later
        big = {}
        load_plan = []

        def emit_loads(interleave_smalls):
            for s in range(NS):
                do_sl = slice(s * DOS, (s + 1) * DOS)
                for nm, iv in (("lw", lw_iv), ("uw", uw_iv)):
                    t = bigp.tile([P, DL * DOS], f32, name=f"{nm}{s}_t")
                    big[(nm, s)] = t
                    tv = t.rearrange("p (dl do) -> p dl do", do=DOS)
                    for c in range(CCH):
                        dl_sl = slice(c * DLC, (c + 1) * DLC)
                        load_plan.append((tv[:, dl_sl, :], iv[:, dl_sl, do_sl]))
            # first big load goes out immediately; the small constant loads
            # ride behind it (they are only needed ~13us in), then the rest.
            nc.sync.dma_start(out=load_plan[0][0], in_=load_plan[0][1])
            smalls = emit_smalls() if interleave_smalls else None
            for out_ap, in_ap in load_plan[1:]:
                nc.sync.dma_start(out=out_ap, in_=in_ap)
            return smalls

        # ---- emission helpers (Tile schedules per-engine streams in
        # roughly emission order, so order matters for overlap) ----

        def phase1_chunk(s, nm, ssq_ps, c):
            """square -> dl-reduce -> dh-reduce matmul for one chunk."""
            x_t = big[(nm, s)]
            sq = sqp.tile([P, FC], f32, name="sq", tag="sq")
            nc.scalar.square(sq, x_t[:, c * FC:(c + 1) * FC])
            part = partp.tile([P, DOS], f32, name="part", tag="part")
            nc.vector.tensor_reduce(
                out=part[:, :],
                in_=sq.rearrange("p (dl do) -> p do dl", do=DOS),
                axis=mybir.AxisListType.X,
                op=Alu.add,
            )
            nc.tensor.matmul(
                ssq_ps[nm][:, :],
                sel1_t[:, :],
                part[:, :],
                start=(c == 0),
                stop=(c == CCH - 1),
            )

        def make_ssq(s):
            return {
                nm: psp.tile([L_LOC, DOS], f32, name=f"ssq_{nm}{s}", tag="ssq", bufs=4)
                for nm in ("lw", "uw")
            }

        # phase-2 engine split: Pool (gpsimd) takes ~1/3 of chunks off DVE
        POOL_TT = {
            pair
            for i, pair in enumerate(
                (n, c) for n in ("lw", "uw") for c in range(CCH)
            )
            if i % 3 == 1
        }

        def phase2_chunk(s, nm, c_t, c):
            do_sl = slice(s * DOS, (s + 1) * DOS)
            ov = lw_ov if nm == "lw" else uw_ov
            x_t = big[(nm, s)]
            xv = x_t.rearrange("p (dl do) -> p dl do", do=DOS)
            chunk = xv[:, c * DLC:(c + 1) * DLC, :]
            bcast = c_t[:, None, :].to_broadcast([P, DLC, DOS])
            eng = nc.gpsimd if (nm, c) in POOL_TT else nc.vector
            eng.tensor_tensor(chunk, chunk, bcast, Alu.mult)
            store_eng.dma_start(out=ov[:, c * DLC:(c + 1) * DLC, do_sl], in_=chunk)

        def coeff_phase(s, ssq_ps):
            do_sl = slice(s * DOS, (s + 1) * DOS)

            # coefficient phase, [L_LOC, DOS] tiles
            def ctile(nm):
                return coefp.tile([L_LOC, DOS], f32, name=nm, tag=nm)

            s_l = ctile("s_l")
            nc.scalar.sqrt(s_l, ssq_ps["lw"][:, :])
            s_u = ctile("s_u")
            nc.scalar.sqrt(s_u, ssq_ps["uw"][:, :])
            es_l = ctile("es_l")
            nc.vector.tensor_scalar_mul(es_l, in0=s_l, scalar1=float(eps_s))
            es_u = ctile("es_u")
            nc.vector.tensor_scalar_mul(es_u, in0=s_u, scalar1=float(eps_s))
            l_t = ctile("l_t")
            nc.vector.tensor_sub(l_t, lb_t[:, do_sl], es_l)
            u_t = ctile("u_t")
            nc.vector.tensor_add(u_t, ub_t[:, do_sl], es_u)

            mp_t = ctile("mp_t")
            nc.vector.tensor_scalar(
                mp_t, in0=l_t, scalar1=0.0, scalar2=None, op0=Alu.is_gt
            )
            mn_t = ctile("mn_t")
            nc.vector.tensor_scalar(
                mn_t, in0=u_t, scalar1=0.0, scalar2=None, op0=Alu.is_lt
            )
            msum = ctile("msum")
            nc.vector.tensor_add(msum, mp_t, mn_t)
            mb_t = ctile("mb_t")
            nc.vector.tensor_scalar(
                mb_t, in0=msum, scalar1=-1.0, scalar2=1.0, op0=Alu.mult, op1=Alu.add
            )

            den = ctile("den")
            nc.vector.tensor_sub(den, u_t, l_t)
            den2 = ctile("den2")
            nc.vector.tensor_scalar(
                den2, in0=den, scalar1=float(EPS), scalar2=None, op0=Alu.add
            )
            rec = ctile("rec")
            nc.vector.reciprocal(rec, den2)
            k_u = ctile("k_u")
            nc.vector.tensor_mul(k_u, u_t, rec)
            kp_u = ctile("kp_u")
            nc.vector.tensor_scalar(
                kp_u, in0=k_u, scalar1=0.0, scalar2=None, op0=Alu.is_gt
            )

            absu = ctile("absu")
            nc.scalar.activation(absu, u_t, mybir.ActivationFunctionType.Abs)
            absl = ctile("absl")
            nc.scalar.activation(absl, l_t, mybir.ActivationFunctionType.Abs)
            k_l = ctile("k_l")
            nc.vector.tensor_tensor(k_l, absu, absl, Alu.is_gt)

            # a = mask_pos + mask_both*k_l ; b = mask_pos + (mask_both*kp_u)*k_u
            mbkl = ctile("mbkl")
            nc.gpsimd.tensor_mul(mbkl, mb_t, k_l)
            a_t = ctile("a_t")
            nc.gpsimd.tensor_add(a_t, mp_t, mbkl)
            mbkp = ctile("mbkp")
            nc.vector.tensor_mul(mbkp, mb_t, kp_u)
            b2_t = ctile("b2_t")
            nc.vector.tensor_mul(b2_t, mbkp, k_u)
            b_t = ctile("b_t")
            nc.vector.tensor_add(b_t, mp_t, b2_t)

            # broadcast a,b -> [P, DOS] via PE, copy PSUM->SBUF on ACT
            ps_a = psp.tile([P, DOS], f32, name="ps_a", tag="bc", bufs=2)
            nc.tensor.matmul(ps_a[:, :], sel2_t[:, :], a_t[:, :], start=True, stop=True)
            A_t = coefp.tile([P, DOS], f32, name="A_t", tag="A_t")
            nc.scalar.copy(A_t, ps_a[:, :])
            ps_b = psp.tile([P, DOS], f32, name="ps_b", tag="bc", bufs=2)
            nc.tensor.matmul(ps_b[:, :], sel2_t[:, :], b_t[:, :], start=True, stop=True)
            B_t = coefp.tile([P, DOS], f32, name="B_t", tag="B_t")
            nc.scalar.copy(B_t, ps_b[:, :])

            # lb_o = mask_pos*lb + (mask_both*k_l)*lb
            q2 = ctile("q2")
            nc.gpsimd.tensor_mul(q2, mbkl, lb_t[:, do_sl])
            q3 = ctile("q3")
            nc.gpsimd.tensor_mul(q3, mp_t, lb_t[:, do_sl])
            lb_o = ctile("lb_o")
            nc.gpsimd.tensor_add(lb_o, q2, q3)
            store_eng.dma_start(out=lb_out[:, s * DOS:(s + 1) * DOS], in_=lb_o)

            # ub_o = mask_pos*ub + mask_both*(kp_u*((ub-l)*k_u))
            t1 = ctile("t1")
            nc.gpsimd.tensor_sub(t1, ub_t[:, do_sl], l_t)
            t2 = ctile("t2")
            nc.gpsimd.tensor_mul(t2, t1, k_u)
            t3 = ctile("t3")
            nc.gpsimd.tensor_mul(t3, kp_u, t2)
            t4 = ctile("t4")
            nc.gpsimd.tensor_mul(t4, mb_t, t3)
            t5 = ctile("t5")
            nc.gpsimd.tensor_mul(t5, mp_t, ub_t[:, do_sl])
            ub_o = ctile("ub_o")
            nc.gpsimd.tensor_add(ub_o, t4, t5)
            store_eng.dma_start(out=ub_out[:, s * DOS:(s + 1) * DOS], in_=ub_o)

            return A_t, B_t

        # ---- driver: pipeline the two splits so split-1's phase-1 runs
        # (on DVE, in emission order) interleaved with split-0's phase-2 ----
        sel1_t = sel2_t = lb_t = ub_t = None
        if reps > 1:
            sel1_t, sel2_t, lb_t, ub_t = emit_smalls()
        loop_cm = tc.For_i(0, reps, 1) if reps > 1 else nullcontext()

        with loop_cm:
            smalls = emit_loads(interleave_smalls=(reps == 1))
            if smalls is not None:
                sel1_t, sel2_t, lb_t, ub_t = smalls

            ssq0 = make_ssq(0)
            for nm in ("lw", "uw"):
                for c in range(CCH):
                    phase1_chunk(0, nm, ssq0, c)
            A0, B0 = coeff_phase(0, ssq0)

            ssq1 = make_ssq(1) if NS > 1 else None
            for nm, c_t in (("lw", A0), ("uw", B0)):
                for c in range(CCH):
                    phase2_chunk(0, nm, c_t, c)
                    if NS > 1:
                        phase1_chunk(1, nm, ssq1, c)

            if NS > 1:
                A1, B1 = coeff_phase(1, ssq1)
                for nm, c_t in (("lw", A1), ("uw", B1)):
                    for c in range(CCH):
                        phase2_chunk(1, nm, c_t, c)

    nc.finalize()
    return nc


def _sel_matrices():
    sel1 = np.zeros((P, L_LOC), np.float32)
    sel1[np.arange(P), np.arange(P) // DH] = 1.0
    sel2 = np.zeros((L_LOC, P), np.float32)
    sel2[np.arange(P) // DH, np.arange(P)] = 1.0
    return sel1, sel2


def _kernel_numpy_fallback(p, eps, lw, lb, uw, ub):
    # reference math in numpy (used only for p != 2)
    q = np.inf if p == 1 else 1.0 / (1.0 - 1.0 / p)
    eps_s = np.float32(eps[0, 0])
    if np.isinf(q):
        nl = np.abs(lw).max(axis=-2)
        nu = np.abs(uw).max(axis=-2)
    else:
        nl = np.sum(np.abs(lw) ** q, axis=-2) ** (1.0 / q)
        nu = np.sum(np.abs(uw) ** q, axis=-2) ** (1.0 / q)
    l = lb - eps_s * nl
    u = ub + eps_s * nu
    mask_pos = (l > 0).astype(np.float32)
    mask_neg = (u < 0).astype(np.float32)
    mask_both = 1.0 - mask_pos - mask_neg
    k_u = u / (u - l + np.float32(EPS))
    kp_u = (k_u > 0).astype(np.float32)
    kn_u = 1.0 - kp_u
    k_l = (np.abs(u) > np.abs(l)).astype(np.float32)
    a = mask_pos + mask_both * k_l
    lw_o = a[:, :, None, :] * lw
    lb_o = mask_pos * lb + (mask_both * k_l) * lb
    uw_o = mask_pos[:, :, None, :] * uw + mask_both[:, :, None, :] * (
        (kp_u[:, :, None, :] * uw + kn_u[:, :, None, :] * lw) * k_u[:, :, None, :]
    )
    ub_o = mask_pos * ub + mask_both * (
        kp_u * ((ub - l) * k_u) + kn_u * ((lb - l) * k_u)
    )
    return lw_o.astype(np.float32), lb_o.astype(np.float32), uw_o.astype(
        np.float32
    ), ub_o.astype(np.float32)


LAST_RESULT = None  # BassKernelResults of the last device run (for test.py)


def kernel(p, eps, lw, lb, uw, ub):
    import os

    p_val = int(np.asarray(p).reshape(-1)[0]) if np.asarray(p).size else 2
    eps_np = np.asarray(eps, np.float32)
    lw = np.asarray(lw, np.float32)
    uw = np.asarray(uw, np.float32)
    lb = np.asarray(lb, np.float32)
    ub = np.asarray(ub, np.float32)

    if p_val != 2:
        return _kernel_numpy_fallback(p_val, eps_np, lw, lb, uw, ub)

    from concourse.bass_utils import run_bass_kernel_spmd

    eps_s = float(eps_np[0, 0])
    nc = _build_nc(eps_s)

    sel1, sel2 = _sel_matrices()
    assert lw.shape == (1, L, DI, DO), lw.shape

    in_maps = []
    for i in range(N_CORES):
        sl = slice(i * L_LOC, (i + 1) * L_LOC)
        in_maps.append(
            {
                "lw_in": np.ascontiguousarray(lw[0, sl]).reshape(P, FREE),
                "uw_in": np.ascontiguousarray(uw[0, sl]).reshape(P, FREE),
                "lb_in": np.ascontiguousarray(lb[0, sl]),
                "ub_in": np.ascontiguousarray(ub[0, sl]),
                "sel1_in": sel1,
                "sel2_in": sel2,
            }
        )

    trace = bool(int(os.environ.get("KERNEL_TRACE", "0")))
    res = run_bass_kernel_spmd(
        nc, in_maps, core_ids=list(range(N_CORES)), trace=trace
    )
    global LAST_RESULT
    LAST_RESULT = res

    lw_o = np.empty((1, L, DI, DO), np.float32)
    uw_o = np.empty((1, L, DI, DO), np.float32)
    lb_o = np.empty((1, L, DO), np.float32)
    ub_o = np.empty((1, L, DO), np.float32)
    for i, r in enumerate(res.results):
        sl = slice(i * L_LOC, (i + 1) * L_LOC)
        lw_o[0, sl] = r["lw_out"].reshape(L_LOC, DI, DO)
        uw_o[0, sl] = r["uw_out"].reshape(L_LOC, DI, DO)
        lb_o[0, sl] = r["lb_out"]
        ub_o[0, sl] = r["ub_out"]
    return lw_o, lb_o, uw_o, ub_o
